# revision 5
# baseline (speedup 1.0000x reference)
"""GAT temperature-scaling kernel for trn2, 8 NeuronCores.

Strategy (edge-parallel, dst-range sharding, per sharding hint):
  Launch 1 (device): per-core PE matmul computes node tables
      (a_src, a_dst, h) = logits @ [W*att_src | W*att_dst | W]   (channel-major out)
  Host re-shard: gathers per-edge src node features (a_src|h, 64B rows) into a
      dst-sorted, degree-bucketed, column-uniform padded slot layout per device
      ("each device holds an edge partition plus gathered src/dst node features").
      Pure indexing - no float math on host.
  Launch 2 (device): pure streaming - sequential slot reads, DVE/ACT edge math
      ex = exp(leaky_relu(a_src[src]+a_dst[dst])), free-dim segmented reductions
      for softmax denom/numerator, then temperature = relu(gat_out@lin_w+b)+1 and
      out = logits / temperature.  No gathers/scatters on device.
"""
import math

import numpy as np

import concourse.bacc as bacc
import concourse.bass as bass
import concourse.tile as tile
from concourse import mybir
from concourse.bass_utils import run_bass_kernel_spmd

P = 128
NCORES = 8
NEG_SLOPE = 0.2
CAP = 384          # max slots per partition per chunk
DUMMY_NEG = -1.0e4


# ---------------------------------------------------------------- host prep
def _plan(edge_index, n_nodes):
    """Build the per-device slot layout (indices only)."""
    n_loc = n_nodes // NCORES
    npad = ((n_loc + P - 1) // P) * P
    nblk = npad // P

    src = edge_index[0].astype(np.int64)
    dst = edge_index[1].astype(np.int64)
    loops = np.arange(n_nodes, dtype=np.int64)
    src = np.concatenate([src, loops])
    dst = np.concatenate([dst, loops])

    dev = dst // n_loc
    order_d, rank_edges = [], []
    deg_sorted_all = np.zeros((NCORES, npad), dtype=np.int64)
    for d in range(NCORES):
        m = dev == d
        es, et = src[m], dst[m] - d * n_loc
        deg = np.bincount(et, minlength=n_loc)
        order = np.argsort(-deg, kind="stable")     # rank -> local node
        rank_of = np.empty(n_loc, dtype=np.int64)
        rank_of[order] = np.arange(n_loc)
        deg_sorted_all[d, :n_loc] = deg[order]
        deg_sorted_all[d, n_loc:] = 1               # pad nodes: 1 slot (dummy2)
        order_d.append(order)
        rank_edges.append((es, rank_of[et]))

    # block widths: max over devices, per 128-rank block (ranks sorted desc)
    w_blk = deg_sorted_all.reshape(NCORES, nblk, P).max(axis=(0, 2))
    w_blk = np.maximum(w_blk, 1)

    # chunks of uniform W
    chunks = []   # (b0, b1, W, soff)
    soff = 0
    b0 = 0
    while b0 < nblk:
        w = int(w_blk[b0])
        b1 = b0 + 1
        while b1 < nblk and (b1 - b0 + 1) * w <= CAP:
            b1 += 1
        chunks.append((b0, b1, w, soff))
        soff += (b1 - b0) * w
        b0 = b1
    totw = soff

    # per-blk slot column offset and width
    blk_w = np.zeros(nblk, dtype=np.int64)
    blk_off = np.zeros(nblk, dtype=np.int64)
    for (b0, b1, w, so) in chunks:
        for b in range(b0, b1):
            blk_w[b] = w
            blk_off[b] = so + (b - b0) * w

    # slot arrays
    dummy1 = n_nodes        # ex = 0 filler
    dummy2 = n_nodes + 1    # zeros row (pad nodes)
    src_arr = np.full((NCORES, P, totw), dummy1, dtype=np.int64)
    for d in range(NCORES):
        es, er = rank_edges[d]     # src global id, dst rank
        o = np.argsort(er, kind="stable")
        es, er = es[o], er[o]
        starts = np.searchsorted(er, np.arange(len(er)))  # not needed; use diff
        # position within each rank group
        grp_start = np.searchsorted(er, er)               # first occurrence index
        j = np.arange(len(er)) - grp_start
        blk = er // P
        p = er % P
        col = blk_off[blk] + j
        src_arr[d, p, col] = es
        # pad nodes: one dummy2 slot
        for r in range(n_loc, npad):
            src_arr[d, r % P, blk_off[r // P]] = dummy2

    return dict(n_loc=n_loc, npad=npad, nblk=nblk, chunks=chunks, totw=totw,
                order_d=order_d, src_arr=src_arr)


# ---------------------------------------------------------------- launch 1
def _build_l1(n_loc, c_in, h8):
    """Per-core: tab24[24, n_loc] = [a_src; a_dst; h] (channel-major).

    PE fp32 matmul truncates operands to ~bf16, so use a 3-pass hi/lo bf16
    split for full fp32 accuracy."""
    nc = bacc.Bacc("TRN2", target_bir_lowering=False, debug=False)
    bf16 = mybir.dt.bfloat16
    lgTh = nc.dram_tensor("lgTh", [c_in, n_loc], bf16, kind="ExternalInput")
    lgTl = nc.dram_tensor("lgTl", [c_in, n_loc], bf16, kind="ExternalInput")
    w_in = nc.dram_tensor("w", [c_in, h8], mybir.dt.float32, kind="ExternalInput")
    asrc = nc.dram_tensor("asrc", [1, h8], mybir.dt.float32, kind="ExternalInput")
    adst = nc.dram_tensor("adst", [1, h8], mybir.dt.float32, kind="ExternalInput")
    tab = nc.dram_tensor("tab", [3 * h8, n_loc], mybir.dt.float32, kind="ExternalOutput")

    cn = 512
    nchunk = math.ceil(n_loc / cn)
    with tile.TileContext(nc) as tc:
        with tc.tile_pool(name="s", bufs=1) as sp, \
             tc.tile_pool(name="io", bufs=3) as iop, \
             tc.tile_pool(name="ps", bufs=2, space="PSUM") as pp:
            wt = sp.tile([c_in, h8], mybir.dt.float32)
            nc.sync.dma_start(out=wt[:], in_=w_in[:])
            at = sp.tile([c_in, h8], mybir.dt.float32)
            nc.sync.dma_start(out=at[:], in_=bass.AP(
                tensor=asrc[:].tensor, offset=asrc[:].offset,
                ap=[[0, c_in], [1, h8]]))
            dt_ = sp.tile([c_in, h8], mybir.dt.float32)
            nc.sync.dma_start(out=dt_[:], in_=bass.AP(
                tensor=adst[:].tensor, offset=adst[:].offset,
                ap=[[0, c_in], [1, h8]]))
            wcat = sp.tile([c_in, 3 * h8], mybir.dt.float32)
            nc.vector.tensor_mul(out=wcat[:, 0:h8], in0=wt[:], in1=at[:])
            nc.vector.tensor_mul(out=wcat[:, h8:2 * h8], in0=wt[:], in1=dt_[:])
            nc.vector.tensor_copy(out=wcat[:, 2 * h8:3 * h8], in_=wt[:])
            # hi/lo bf16 split of wcat
            wc_hi = sp.tile([c_in, 3 * h8], bf16)
            nc.vector.tensor_copy(out=wc_hi[:], in_=wcat[:])
            wc_h32 = sp.tile([c_in, 3 * h8], mybir.dt.float32)
            nc.vector.tensor_copy(out=wc_h32[:], in_=wc_hi[:])
            wc_lf = sp.tile([c_in, 3 * h8], mybir.dt.float32)
            nc.vector.tensor_sub(out=wc_lf[:], in0=wcat[:], in1=wc_h32[:])
            wc_lo = sp.tile([c_in, 3 * h8], bf16)
            nc.vector.tensor_copy(out=wc_lo[:], in_=wc_lf[:])

            for i in range(nchunk):
                c0 = i * cn
                c1 = min(c0 + cn, n_loc)
                w_ = c1 - c0
                lgh = iop.tile([c_in, cn], bf16, tag="lgh")
                nc.sync.dma_start(out=lgh[:, :w_], in_=lgTh[:, c0:c1])
                lgl = iop.tile([c_in, cn], bf16, tag="lgl")
                nc.sync.dma_start(out=lgl[:, :w_], in_=lgTl[:, c0:c1])
                ps = pp.tile([3 * h8, cn], mybir.dt.float32, tag="ps")
                nc.tensor.matmul(out=ps[:, :w_], lhsT=wc_hi[:], rhs=lgh[:, :w_],
                                 start=True, stop=False)
                nc.tensor.matmul(out=ps[:, :w_], lhsT=wc_lo[:], rhs=lgh[:, :w_],
                                 start=False, stop=False)
                nc.tensor.matmul(out=ps[:, :w_], lhsT=wc_hi[:], rhs=lgl[:, :w_],
                                 start=False, stop=True)
                ob = iop.tile([3 * h8, cn], mybir.dt.float32, tag="ob")
                nc.scalar.copy(out=ob[:, :w_], in_=ps[:, :w_])
                nc.sync.dma_start(out=tab[:, c0:c1], in_=ob[:, :w_])
    nc.compile()
    return nc


# ---------------------------------------------------------------- launch 2
def _build_l2(nblk, chunks, totw, h8, c_in):
    nc = bacc.Bacc("TRN2", target_bir_lowering=False, debug=False)
    f32 = mybir.dt.float32
    slots = nc.dram_tensor("slots", [P, totw * 2 * h8], f32, kind="ExternalInput")
    dtab = nc.dram_tensor("dtab", [P, nblk * h8], f32, kind="ExternalInput")
    lgs = nc.dram_tensor("lgs", [P, nblk * c_in], f32, kind="ExternalInput")
    gbias = nc.dram_tensor("gbias", [1, h8], f32, kind="ExternalInput")
    linw = nc.dram_tensor("linw", [1, h8], f32, kind="ExternalInput")
    linb = nc.dram_tensor("linb", [1, 1], f32, kind="ExternalInput")
    tbias = nc.dram_tensor("tbias", [1, 1], f32, kind="ExternalInput")
    outd = nc.dram_tensor("outd", [P, nblk * c_in], f32, kind="ExternalOutput")

    F = 2 * h8  # 16 floats per slot

    with tile.TileContext(nc) as tc:
        with tc.tile_pool(name="s", bufs=1) as sp, \
             tc.tile_pool(name="g", bufs=2) as gp, \
             tc.tile_pool(name="t", bufs=2) as tp, \
             tc.tile_pool(name="r", bufs=3) as rp, \
             tc.tile_pool(name="fin", bufs=3) as fp:
            dt_ = sp.tile([P, nblk * h8], f32)
            nc.sync.dma_start(out=dt_[:], in_=dtab[:])
            den = sp.tile([P, nblk * h8], f32)
            num = sp.tile([P, nblk * h8], f32)
            nc.vector.memset(den[:], 0.0)
            nc.vector.memset(num[:], 0.0)

            for (b0, b1, w, soff) in chunks:
                nb = b1 - b0
                ns = nb * w
                sl = gp.tile([P, ns * F], f32, tag="sl")
                nc.sync.dma_start(out=sl[:], in_=slots[:, soff * F:(soff + ns) * F])
                s4 = sl[:].rearrange("p (b w f) -> p b w f", b=nb, f=F)
                t = tp.tile([P, ns * h8], f32, tag="t")
                t4 = t[:].rearrange("p (b w c) -> p b w c", b=nb, c=h8)
                dv = dt_[:].rearrange("p (b c) -> p b c", c=h8)[:, b0:b1, :]
                dbc = bass.AP(tensor=dv.tensor, offset=dv.offset,
                              ap=[dv.ap[0], dv.ap[1], [0, w], dv.ap[2]])
                # t = s + d
                nc.vector.tensor_add(out=t4, in0=s4[:, :, :, 0:h8], in1=dbc)
                # t = exp(leaky_relu(t))  (Lrelu HW table has fixed slope ->
                # compute leaky as max(t, 0.2*t) explicitly)
                u = tp.tile([P, ns * h8], f32, tag="xh")
                nc.scalar.mul(out=u[:], in_=t[:], mul=NEG_SLOPE)
                nc.vector.tensor_tensor(out=t[:], in0=t[:], in1=u[:],
                                        op=mybir.AluOpType.max)
                nc.scalar.activation(out=t[:], in_=t[:],
                                     func=mybir.ActivationFunctionType.Exp)
                # xh = ex * h
                xh = tp.tile([P, ns * h8], f32, tag="xh")
                xh4 = xh[:].rearrange("p (b w c) -> p b w c", b=nb, c=h8)
                nc.vector.tensor_mul(out=xh4, in0=t4, in1=s4[:, :, :, h8:F])
                # reduce over w (strided innermost)
                for (acc, buf) in ((den, t), (num, xh)):
                    b4 = buf[:].rearrange("p (b w c) -> p b w c", b=nb, c=h8)
                    rin = bass.AP(tensor=b4.tensor, offset=b4.offset,
                                  ap=[b4.ap[0], b4.ap[1], b4.ap[3], b4.ap[2]])
                    rt = rp.tile([P, nb * h8], f32, tag="rt")
                    nc.vector.tensor_reduce(out=rt[:], in_=rin,
                                            axis=mybir.AxisListType.X,
                                            op=mybir.AluOpType.add)
                    accs = acc[:, b0 * h8:b1 * h8]
                    nc.vector.tensor_add(out=accs, in0=accs, in1=rt[:])

            # ---- node phase
            gb = sp.tile([P, h8], f32)
            nc.sync.dma_start(out=gb[:], in_=bass.AP(
                tensor=gbias[:].tensor, offset=gbias[:].offset,
                ap=[[0, P], [1, h8]]))
            lw = sp.tile([P, h8], f32)
            nc.sync.dma_start(out=lw[:], in_=bass.AP(
                tensor=linw[:].tensor, offset=linw[:].offset,
                ap=[[0, P], [1, h8]]))
            lb = sp.tile([P, 1], f32)
            nc.sync.dma_start(out=lb[:], in_=linb[:].to_broadcast([P, 1]))
            tb = sp.tile([P, 1], f32)
            nc.sync.dma_start(out=tb[:], in_=tbias[:].to_broadcast([P, 1]))

            rden = sp.tile([P, nblk * h8], f32)
            nc.vector.reciprocal(out=rden[:], in_=den[:])
            gat = sp.tile([P, nblk * h8], f32)
            nc.vector.tensor_mul(out=gat[:], in0=num[:], in1=rden[:])
            gbb = bass.AP(tensor=gb[:].tensor, offset=gb[:].offset,
                          ap=[gb[:].ap[0], [0, nblk], [1, h8]])
            g3 = gat[:].rearrange("p (b c) -> p b c", c=h8)
            nc.vector.tensor_add(out=g3, in0=g3, in1=gbb)
            # tl = sum_c gat*lin_w
            lwb = bass.AP(tensor=lw[:].tensor, offset=lw[:].offset,
                          ap=[lw[:].ap[0], [0, nblk], [1, h8]])
            gl = rden  # rden is dead after gat; reuse its slot
            nc.vector.tensor_mul(out=gl[:].rearrange("p (b c) -> p b c", c=h8),
                                 in0=g3, in1=lwb)
            tl = sp.tile([P, nblk], f32)
            nc.vector.tensor_reduce(out=tl[:],
                                    in_=gl[:].rearrange("p (b c) -> p b c", c=h8),
                                    axis=mybir.AxisListType.X,
                                    op=mybir.AluOpType.add)
            # temp = relu(tl + lin_b) + temp_bias ; inv = 1/temp
            nc.scalar.activation(out=tl[:], in_=tl[:],
                                 func=mybir.ActivationFunctionType.Relu,
                                 bias=lb[:], scale=1.0)
            nc.vector.tensor_scalar(out=tl[:], in0=tl[:], scalar1=tb[:],
                                    scalar2=None, op0=mybir.AluOpType.add)
            inv = sp.tile([P, nblk], f32)
            nc.vector.reciprocal(out=inv[:], in_=tl[:])

            # out = logits_sorted * inv  (stream in block chunks)
            bstep = 28
            for c0 in range(0, nblk, bstep):
                c1 = min(c0 + bstep, nblk)
                nb = c1 - c0
                lgt = fp.tile([P, bstep * c_in], f32, tag="lgt")
                nc.sync.dma_start(out=lgt[:, :nb * c_in],
                                  in_=lgs[:, c0 * c_in:c1 * c_in])
                iv = inv[:, c0:c1]
                ivb = bass.AP(tensor=iv.tensor, offset=iv.offset,
                              ap=[iv.ap[0], iv.ap[1], [0, c_in]])
                ot = fp.tile([P, bstep * c_in], f32, tag="ot")
                nc.vector.tensor_mul(
                    out=ot[:, :nb * c_in].rearrange("p (b c) -> p b c", c=c_in),
                    in0=lgt[:, :nb * c_in].rearrange("p (b c) -> p b c", c=c_in),
                    in1=ivb)
                nc.sync.dma_start(out=outd[:, c0 * c_in:c1 * c_in],
                                  in_=ot[:, :nb * c_in])
    nc.compile()
    return nc


# ---------------------------------------------------------------- driver
def _run(nc, in_maps):
    res = run_bass_kernel_spmd(nc, in_maps, core_ids=list(range(NCORES)))
    return res.results


def kernel(logits, edge_index, gat_w, att_src, att_dst, gat_bias, lin_w, lin_b,
           temp_bias):
    logits = np.asarray(logits, dtype=np.float32)
    n, c_in = logits.shape
    h8 = gat_w.shape[1]
    plan = _plan(np.asarray(edge_index), n)
    n_loc, npad, nblk = plan["n_loc"], plan["npad"], plan["nblk"]
    chunks, totw = plan["chunks"], plan["totw"]

    # ---- launch 1
    import ml_dtypes
    nc1 = _build_l1(n_loc, c_in, h8)
    lgT = np.ascontiguousarray(logits.T)
    lgT_hi = lgT.astype(ml_dtypes.bfloat16)
    lgT_lo = (lgT - lgT_hi.astype(np.float32)).astype(ml_dtypes.bfloat16)
    in1 = [{"lgTh": np.ascontiguousarray(lgT_hi[:, d * n_loc:(d + 1) * n_loc]),
            "lgTl": np.ascontiguousarray(lgT_lo[:, d * n_loc:(d + 1) * n_loc]),
            "w": np.asarray(gat_w, np.float32),
            "asrc": np.asarray(att_src, np.float32).reshape(1, h8),
            "adst": np.asarray(att_dst, np.float32).reshape(1, h8)}
           for d in range(NCORES)]
    r1 = _run(nc1, in1)
    tab24 = np.concatenate([r1[d]["tab"] for d in range(NCORES)], axis=1)  # [24, n]
    a_src = tab24[0:h8].T          # [n, 8]
    a_dst = tab24[h8:2 * h8].T
    h_all = tab24[2 * h8:3 * h8].T

    # node feature table with dummy rows (indexing only)
    table16 = np.zeros((n + 2, 2 * h8), dtype=np.float32)
    table16[:n, 0:h8] = a_src
    table16[:n, h8:] = h_all
    table16[n, 0:h8] = DUMMY_NEG

    # ---- host re-shard (gather per-edge src features into slot layout)
    in2 = []
    for d in range(NCORES):
        sa = plan["src_arr"][d]                      # [128, totw]
        ex = table16[sa.ravel()].reshape(P, totw * 2 * h8)
        order = plan["order_d"][d]
        # rank r=blk*128+p -> global node order[r]; pads -> zeros
        ranks = np.arange(npad)
        gnode = np.full(npad, -1, dtype=np.int64)
        gnode[:n_loc] = order[:n_loc] + d * n_loc
        dt_ = np.zeros((npad, h8), dtype=np.float32)
        lgsort = np.zeros((npad, c_in), dtype=np.float32)
        valid = gnode >= 0
        dt_[valid] = a_dst[gnode[valid]]
        lgsort[valid] = logits[gnode[valid]]
        # [npad, X] with rank = blk*128 + p  ->  [p, blk, X]
        dt_ = dt_.reshape(nblk, P, h8).transpose(1, 0, 2).reshape(P, nblk * h8)
        lgsort = lgsort.reshape(nblk, P, c_in).transpose(1, 0, 2).reshape(P, nblk * c_in)
        in2.append({"slots": np.ascontiguousarray(ex),
                    "dtab": np.ascontiguousarray(dt_),
                    "lgs": np.ascontiguousarray(lgsort),
                    "gbias": np.asarray(gat_bias, np.float32).reshape(1, h8),
                    "linw": np.asarray(lin_w, np.float32).reshape(1, h8),
                    "linb": np.asarray(lin_b, np.float32).reshape(1, 1),
                    "tbias": np.asarray(temp_bias, np.float32).reshape(1, 1)})

    # ---- launch 2
    nc2 = _build_l2(nblk, chunks, totw, h8, c_in)
    r2 = _run(nc2, in2)
    global _last_ncs
    _last_ncs = {"L1": (nc1, in1), "L2": (nc2, in2)}

    # ---- unshard
    out = np.zeros((n, c_in), dtype=np.float32)
    for d in range(NCORES):
        od = r2[d]["outd"].reshape(P, nblk, c_in).transpose(1, 0, 2).reshape(npad, c_in)
        order = plan["order_d"][d]
        out[order + d * n_loc] = od[:n_loc]
    return out
